# revision 7
# baseline (speedup 1.0000x reference)
"""Causal attention kernel for Trainium2 (Bass/Tile), batch-sharded over 8 cores.

Reference computation (per batch b):
    S = Q @ K^T                  [S, S]
    S -= triu(ones, k=1) * 1e10  (causal mask, applied before scaling)
    P = softmax(S / sqrt(512), axis=-1)
    O = P @ V                    [S, D]

Shapes: B=16, S=2048, D=512, fp32. Each of the 8 cores handles 2 batches.

Design notes:
  - All matmuls run as float32r (full-rate PE; fp32 would be 4x slower).
  - S^T layout ([keys, queries]) so the exp output P^T feeds the PV matmul
    directly as the stationary operand; no per-tile transposes of P.
  - No max-subtraction in the softmax: logits after scaling are ~N(0,1)
    (|logit| < ~7), exp cannot overflow in fp32.
  - Row sums of P come from an extra N=1 matmul per (i,j) pair reusing the
    already-loaded P^T stationary block against a ones vector.
  - Q^T / K^T are built on-chip with PE transposes (d must sit on partitions
    for both QK^T operands).
"""

import sys

sys.path.insert(0, "/opt/trn_rl_repo")

from contextlib import ExitStack

import numpy as np

import concourse.bass as bass
import concourse.tile as tile
from concourse import bacc, mybir
from concourse.bass_utils import run_bass_kernel_spmd
from concourse.masks import make_identity

N_CORES = 8
B_FULL = 16
B_LOC = B_FULL // N_CORES  # batches per core
S = 2048
D = 512
P = 128  # partitions
DC = D // P  # d-chunks (4)
NKB = S // P  # key blocks per batch (16)
NG = S // 512  # query groups of 512 (4)
SCALE = 1.0 / np.sqrt(np.float32(D))  # 1/22.627
MASK_VAL = -1e9

F32 = mybir.dt.float32
F32R = mybir.dt.float32r


def _build_attention(ctx: ExitStack, tc: tile.TileContext, out_ap, q_ap, k_ap, v_ap):
    nc = tc.nc

    consts = ctx.enter_context(tc.tile_pool(name="consts", bufs=1))
    stage = ctx.enter_context(tc.tile_pool(name="stage", bufs=4))
    kt_pool = ctx.enter_context(tc.tile_pool(name="kt", bufs=1))
    qt_pool = ctx.enter_context(tc.tile_pool(name="qt", bufs=2))
    v_pool = ctx.enter_context(tc.tile_pool(name="v", bufs=2))
    pt_pool = ctx.enter_context(tc.tile_pool(name="pt", bufs=2))
    o_pool = ctx.enter_context(tc.tile_pool(name="o", bufs=4))
    small = ctx.enter_context(tc.tile_pool(name="small", bufs=4))
    ps_st = ctx.enter_context(tc.tile_pool(name="ps_st", bufs=2, space="PSUM"))
    ps_tp = ctx.enter_context(tc.tile_pool(name="ps_tp", bufs=2, space="PSUM"))
    ps_o = ctx.enter_context(tc.tile_pool(name="ps_o", bufs=2, space="PSUM"))
    ps_sum = ctx.enter_context(tc.tile_pool(name="ps_sum", bufs=2, space="PSUM"))

    # Identity (for PE transpose) and causal mask for diagonal blocks.
    ident = consts.tile([P, P], F32)
    make_identity(nc, ident)
    # S^T orientation: entry [kk, qq] is masked (add -1e9) when kk > qq,
    # i.e. strictly below the diagonal (partition index > free index).
    mask = consts.tile([P, P], F32)
    nc.gpsimd.memset(mask, 0.0)
    nc.gpsimd.affine_select(
        out=mask,
        in_=mask,
        compare_op=mybir.AluOpType.is_ge,
        fill=MASK_VAL,
        base=0,
        # keep 0.0 where (-kk + qq) >= 0, else fill MASK_VAL
        pattern=[[1, P]],
        channel_multiplier=-1,
    )
    # fp32r matmuls need an even moving free dim, so the sums matmul uses a
    # [P, 2] ones operand (column 0 of the result is read, column 1 ignored).
    ones_f = consts.tile([P, 2], F32)
    nc.vector.memset(ones_f, 1.0)
    ones = consts.tile([P, 2], F32R)
    nc.vector.tensor_copy(ones, ones_f)

    for b in range(B_LOC):
        # ---- Stage 0: load V; build K^T via PE transposes -------------------
        v_sb = v_pool.tile([P, NKB, D], F32R)
        nc.gpsimd.dma_start(
            out=v_sb, in_=v_ap[b].rearrange("(kb p) d -> p kb d", p=P)
        )

        kt = kt_pool.tile([P, DC, S], F32R)  # [d_part, dc, keys]
        for kb in range(NKB):
            knat = stage.tile([P, D], F32, tag="nat")
            nc.sync.dma_start(out=knat, in_=k_ap[b, kb * P : (kb + 1) * P, :])
            tp = ps_tp.tile([P, DC, P], F32)
            for dc in range(DC):
                nc.tensor.transpose(
                    tp[:, dc, :], knat[:, dc * P : (dc + 1) * P], ident
                )
            nc.scalar.copy(kt[:, :, kb * P : (kb + 1) * P], tp)

        for g in range(NG):
            # ---- Build Q^T for this query group (512 queries) ---------------
            qt = qt_pool.tile([P, DC, 512], F32R)  # [d_part, dc, q_local]
            for t in range(4):
                qb = 4 * g + t
                qnat = stage.tile([P, D], F32, tag="nat")
                nc.sync.dma_start(
                    out=qnat, in_=q_ap[b, qb * P : (qb + 1) * P, :]
                )
                tp = ps_tp.tile([P, DC, P], F32)
                for dc in range(DC):
                    nc.tensor.transpose(
                        tp[:, dc, :], qnat[:, dc * P : (dc + 1) * P], ident
                    )
                nc.vector.tensor_copy(qt[:, :, t * P : (t + 1) * P], tp)

            # ---- Phase A: S^T = K^T.T @ Q^T per key block; mask; exp --------
            pt = pt_pool.tile([P, NKB, 512], F32R)  # [k_part, j, q_local]
            for j in range(4 * g + 4):
                o_off = max(0, (j - 4 * g) * P)  # first allowed local query
                w = 512 - o_off
                st = ps_st.tile([P, 512], F32)
                for dc in range(DC):
                    nc.tensor.matmul(
                        st[:, :w],
                        kt[:, dc, j * P : (j + 1) * P],
                        qt[:, dc, o_off:512],
                        start=(dc == 0),
                        stop=(dc == DC - 1),
                    )
                if j >= 4 * g:  # diagonal block: in-block causal mask
                    nc.vector.tensor_add(st[:, 0:P], st[:, 0:P], mask)
                nc.scalar.activation(
                    pt[:, j, o_off:512],
                    st[:, :w],
                    mybir.ActivationFunctionType.Exp,
                    bias=0.0,
                    scale=float(SCALE),
                )

            # ---- Phase B: O = P^T.T @ V; sums; normalize; store -------------
            for t in range(4):
                i = 4 * g + t  # global query tile
                o_ps = ps_o.tile([P, D], F32)
                s_ps = ps_sum.tile([P, 2], F32)
                for j in range(i + 1):
                    lhsT = pt[:, j, t * P : (t + 1) * P]
                    nc.tensor.matmul(
                        o_ps, lhsT, v_sb[:, j, :], start=(j == 0), stop=(j == i)
                    )
                    nc.tensor.matmul(
                        s_ps, lhsT, ones, start=(j == 0), stop=(j == i)
                    )
                recip = small.tile([P, 1], F32)
                nc.vector.reciprocal(recip, s_ps[:, 0:1])
                o_sb = o_pool.tile([P, D], F32)
                nc.vector.tensor_scalar_mul(o_sb, o_ps, recip)
                nc.sync.dma_start(
                    out=out_ap[b, i * P : (i + 1) * P, :], in_=o_sb
                )


def build_nc():
    nc = bacc.Bacc(None, target_bir_lowering=False, debug=False)
    q = nc.dram_tensor("query", [B_LOC, S, D], F32, kind="ExternalInput").ap()
    k = nc.dram_tensor("key", [B_LOC, S, D], F32, kind="ExternalInput").ap()
    v = nc.dram_tensor("value", [B_LOC, S, D], F32, kind="ExternalInput").ap()
    out = nc.dram_tensor("out", [B_LOC, S, D], F32, kind="ExternalOutput").ap()
    with tile.TileContext(nc) as tc:
        with ExitStack() as ctx:
            _build_attention(ctx, tc, out, q, k, v)
    nc.compile()
    return nc


def kernel(query, key, value, _trace=False):
    query = np.ascontiguousarray(query, dtype=np.float32)
    key = np.ascontiguousarray(key, dtype=np.float32)
    value = np.ascontiguousarray(value, dtype=np.float32)
    nc = build_nc()
    in_maps = [
        {
            "query": query[c * B_LOC : (c + 1) * B_LOC],
            "key": key[c * B_LOC : (c + 1) * B_LOC],
            "value": value[c * B_LOC : (c + 1) * B_LOC],
        }
        for c in range(N_CORES)
    ]
    res = run_bass_kernel_spmd(nc, in_maps, list(range(N_CORES)), trace=_trace)
    out = np.concatenate([res.results[c]["out"] for c in range(N_CORES)], axis=0)
    if _trace:
        return out, res
    return out
